# revision 1
# baseline (speedup 1.0000x reference)
"""Bass/Trainium2 kernel for BidirectionalAttention (RMSNorm + QKV + RoPE +
non-causal attention + out-proj + residual), distributed over 8 NeuronCores.

Sharding: core c handles batch b = c // 4 and head-group hg = c % 4
(4 of the 16 heads = a 512-wide slice of the qkv/out feature dims).
Each core computes a partial out-projection; the host sums the 4 partials
per batch and adds the residual.

v2: fp8(e4m3) + MatmulPerfMode.DoubleRow on the contraction>=256 matmul
sites (qkv, att@v, softmax-denominator, out-proj) -- 0.5 cycles/row with a
256-deep contraction = 4x the f32r matmul rate.  scores stay bf16
(contraction is only head_dim=128).  Numerics tricks:
  - at = exp(score*scale - 3.5): the bias shift keeps exp outputs inside
    fp8 range (max ~110 < 240) and cancels exactly in softmax.
  - RMSNorm is deferred: QKV matmuls run on raw fp8 x; the 1/rms(t) scale
    is folded into the rope cos/sin tables for q,k (linear in q) and into
    the PSUM->SBUF copy of v via a transposed-sstd per-partition scalar
    (sstd^T obtained with a PE transpose).
  - squares for sum(x^2) are computed from a parallel bf16 copy of x on
    DVE at the 4x_2p rate; the cross-partition sum is a ones-stationary
    bf16 matmul.
  - rope runs on bf16 SBUF tiles at the DVE 4x_2p rate; the rotate-half
    partition shift is a dedicated shift-copy, and the sign lives in a
    host-prepared half-negated sin table (DVE two-SBUF-input ops must
    share a base partition).
"""

import math
import sys

import numpy as np

for _p in ("/opt/trn_rl_repo", "/root/.axon_site/_ro/trn_rl_repo"):
    if _p not in sys.path:
        sys.path.append(_p)

B = 2
T = 2048
D_MODEL = 2048
N_HEADS = 16
HEAD_DIM = 128
EPS = 1e-6
ROPE_BASE = 10000.0
N_CORES = 8
HG = 4                           # head groups (cores per batch)
HEADS_PER_CORE = N_HEADS // HG   # 4
F = HEADS_PER_CORE * HEAD_DIM    # 512 features per core for each of q/k/v
SCALE = 1.0 / math.sqrt(HEAD_DIM)
EXP_SHIFT = -3.5                 # at = exp(score*SCALE + EXP_SHIFT)

KC = D_MODEL // 128   # 16 contraction chunks
KC2 = KC // 2         # 8 DoubleRow chunk pairs
TH = T // 2           # phase-A token half

_PROGRAMS: dict = {}
DEBUG_TAPS = False


def _build_program(reps: int = 1):
    """Build + compile the per-core Bass program (SPMD, identical on all cores)."""
    key = (reps, DEBUG_TAPS)
    if key in _PROGRAMS:
        return _PROGRAMS[key]

    import concourse.bacc as bacc
    import concourse.mybir as mybir
    from concourse import tile
    from concourse.bass import ts

    F32 = mybir.dt.float32
    BF16 = mybir.dt.bfloat16
    FP8 = mybir.dt.float8e4
    Act = mybir.ActivationFunctionType
    DR = mybir.MatmulPerfMode.DoubleRow

    nc = bacc.Bacc("TRN2", target_bir_lowering=False, debug=False,
                   num_devices=N_CORES)

    x8d = nc.dram_tensor("x8", [128, KC, T], FP8, kind="ExternalInput")
    xbd = nc.dram_tensor("xb", [128, KC, T], BF16, kind="ExternalInput")
    wqd = nc.dram_tensor("wq8", [128, KC2, 2, F], FP8, kind="ExternalInput")
    wkd = nc.dram_tensor("wk8", [128, KC2, 2, F], FP8, kind="ExternalInput")
    wvd = nc.dram_tensor("wv8", [128, KC2, 2, F], FP8, kind="ExternalInput")
    wod = nc.dram_tensor("wo8", [128, 2, 2, D_MODEL], FP8, kind="ExternalInput")
    cosd = nc.dram_tensor("cosb", [HEAD_DIM, T], BF16, kind="ExternalInput")
    sinnd = nc.dram_tensor("sinn", [HEAD_DIM, T], BF16, kind="ExternalInput")
    identd = nc.dram_tensor("ident", [128, 128], F32, kind="ExternalInput")
    pod = nc.dram_tensor("po", [D_MODEL, T], BF16, kind="ExternalOutput")
    if DEBUG_TAPS:
        dbg_kr = nc.dram_tensor("dbg_kr", [HEAD_DIM, T], BF16,
                                kind="ExternalOutput")
        dbg_vt = nc.dram_tensor("dbg_vt", [128, KC, F], FP8,
                                kind="ExternalOutput")
        dbg_sT = nc.dram_tensor("dbg_sT", [128, TH // 128], F32,
                                kind="ExternalOutput")
        dbg_den = nc.dram_tensor("dbg_den", [128, 512], F32,
                                 kind="ExternalOutput")
        dbg_at = nc.dram_tensor("dbg_at", [128, 2, 512], FP8,
                                kind="ExternalOutput")

    with tile.TileContext(nc) as tc:
        with tc.tile_pool(name="persist", bufs=1) as persist:
            cos_t = persist.tile([HEAD_DIM, T], BF16, tag="cos")
            sinn_t = persist.tile([HEAD_DIM, T], BF16, tag="sinn")
            ident_t = persist.tile([128, 128], F32, tag="ident")
            nc.sync.dma_start(cos_t[:], cosd.ap())
            nc.sync.dma_start(sinn_t[:], sinnd.ap())
            nc.sync.dma_start(ident_t[:], identd.ap())
            ones_bf = persist.tile([128, 128], BF16, tag="onesb")
            nc.vector.memset(ones_bf[:], 1.0)
            ones8 = persist.tile([128, 2, 128], FP8, tag="ones8")
            nc.vector.memset(ones8[:], 1.0)
            eps_t = persist.tile([128, 1], F32, tag="eps")
            nc.vector.memset(eps_t[:], EPS)
            shift_t = persist.tile([128, 1], F32, tag="shift")
            nc.vector.memset(shift_t[:], EXP_SHIFT)

            # rms-scaled rope tables for this rep (written per half)
            cs_t = persist.tile([HEAD_DIM, T], BF16, tag="cs")
            ss_t = persist.tile([HEAD_DIM, T], BF16, tag="ss")

            qrope = [persist.tile([HEAD_DIM, T], BF16, name=f"qr{h}", tag=f"qr{h}")
                     for h in range(HEADS_PER_CORE)]
            krope = [persist.tile([HEAD_DIM, T], BF16, name=f"kr{h}", tag=f"kr{h}")
                     for h in range(HEADS_PER_CORE)]
            vt = persist.tile([128, KC, F], FP8, tag="vt")

            def emit_body():
                # weights + x resident for the whole body
                with (
                    tc.tile_pool(name="wpool", bufs=1) as wpool,
                    tc.tile_pool(name="pa", bufs=1) as pa,
                ):
                    wq_t = wpool.tile([128, KC2, 2, F], FP8, tag="wq")
                    wk_t = wpool.tile([128, KC2, 2, F], FP8, tag="wk")
                    wv_t = wpool.tile([128, KC2, 2, F], FP8, tag="wv")
                    wo_t = wpool.tile([128, 2, 2, D_MODEL], FP8, tag="wo")
                    x8_t = wpool.tile([128, KC, T], FP8, tag="x8")
                    nc.sync.dma_start(wq_t[:], wqd.ap())
                    nc.sync.dma_start(wk_t[:], wkd.ap())
                    nc.sync.dma_start(wv_t[:], wvd.ap())
                    nc.sync.dma_start(wo_t[:], wod.ap())
                    nc.sync.dma_start(x8_t[:], x8d.ap())

                    # ---------- Phase A: rms-stats + QKV + RoPE ----------
                    for half in range(2):
                        tsl = slice(half * TH, (half + 1) * TH)
                        with tc.tile_pool(name="ph", bufs=1) as ph:
                            with tc.tile_pool(name="ps_ssq", bufs=1,
                                              space="PSUM") as ps_ssq:
                                # squares from a bf16 copy of x (DVE 4x rate)
                                ssq = [ps_ssq.tile([128, 512], F32, tag="ssq",
                                                   bufs=2, name=f"ssq{half}_{j}")
                                       for j in range(2)]
                                for c in range(KC2):
                                    xb_t = ph.tile([128, 2, TH], BF16, tag="xb",
                                                   bufs=4)
                                    nc.scalar.dma_start(xb_t[:],
                                                        xbd[:, 2 * c:2 * c + 2, tsl])
                                    sq_t = ph.tile([128, 2, TH], BF16, tag="sq",
                                                   bufs=4)
                                    nc.scalar.activation(sq_t[:], xb_t[:],
                                                         Act.Square)
                                    for i in range(2):
                                        for j in range(2):
                                            nc.tensor.matmul(
                                                ssq[j][:], ones_bf[:],
                                                sq_t[:, i, ts(j, 512)],
                                                start=(c == 0 and i == 0),
                                                stop=(c == KC2 - 1 and i == 1))

                                # sstd = 1/sqrt(ssq/D + eps), replicated
                                sstd = ph.tile([128, TH], F32, tag="sstd")
                                for j in range(2):
                                    nc.scalar.activation(
                                        sstd[:, ts(j, 512)], ssq[j][:], Act.Sqrt,
                                        bias=eps_t[:], scale=1.0 / D_MODEL)
                                nc.vector.reciprocal(sstd[:], sstd[:])

                                # rms-scaled rope tables
                                nc.vector.tensor_mul(cs_t[:, tsl],
                                                     cos_t[:, tsl], sstd[:])
                                nc.vector.tensor_mul(ss_t[:, tsl],
                                                     sinn_t[:, tsl], sstd[:])

                            # q, k: DoubleRow fp8 matmuls + bf16 rope
                            for grp, dest in (("q", qrope), ("k", krope)):
                                wt = wq_t if grp == "q" else wk_t
                                with tc.tile_pool(name=f"ps_{grp}", bufs=1,
                                                  space="PSUM") as ps_qk:
                                    acc = [[ps_qk.tile([128, 512], F32,
                                                       tag="a", bufs=8,
                                                       name=f"{grp}{half}{e}{j}")
                                            for j in range(2)]
                                           for e in range(4)]
                                    for e in range(4):
                                        for c in range(KC2):
                                            for j in range(2):
                                                nc.tensor.matmul(
                                                    acc[e][j][:],
                                                    wt[:, c, :, ts(e, 128)],
                                                    x8_t[:, 2 * c:2 * c + 2,
                                                         slice(half * TH + j * 512,
                                                               half * TH + (j + 1) * 512)],
                                                    start=(c == 0),
                                                    stop=(c == KC2 - 1),
                                                    perf_mode=DR)
                                        for j in range(2):
                                            csl = slice(half * TH + j * 512,
                                                        half * TH + (j + 1) * 512)
                                            ps = acc[e][j]
                                            qtmp = ph.tile([128, 512], BF16,
                                                           tag="qtmp", bufs=3)
                                            nc.scalar.activation(qtmp[:], ps[:],
                                                                 Act.Copy)
                                            qrot = ph.tile([128, 512], BF16,
                                                           tag="qrot", bufs=3)
                                            nc.vector.tensor_copy(
                                                qrot[0:64, :], qtmp[64:128, :])
                                            nc.vector.tensor_copy(
                                                qrot[64:128, :], qtmp[0:64, :])
                                            rt = ph.tile([128, 512], BF16,
                                                         tag="rt", bufs=2)
                                            nc.gpsimd.tensor_mul(
                                                rt[:], qrot[:], ss_t[:, csl])
                                            rc = ph.tile([128, 512], BF16,
                                                         tag="rc", bufs=2)
                                            nc.vector.tensor_mul(
                                                rc[:], qtmp[:], cs_t[:, csl])
                                            nc.vector.tensor_add(
                                                dest[e][:, csl], rt[:], rc[:])

                            # transposed sstd (per-token scalar for v)
                            sstdT = ph.tile([128, TH // 128], F32, tag="sstdT")
                            with tc.tile_pool(name="ps_tp", bufs=1,
                                              space="PSUM") as ps_tp:
                                tp_ps = ps_tp.tile([128, TH // 128, 128], F32,
                                                   tag="tp")
                                for u in range(TH // 128):
                                    nc.tensor.matmul(tp_ps[:, u, :],
                                                     sstd[:, ts(u, 128)],
                                                     ident_t[:],
                                                     is_transpose=True)
                                nc.vector.tensor_copy(sstdT[:], tp_ps[:, :, 0:1])
                            if DEBUG_TAPS and half == 0:
                                nc.sync.dma_start(dbg_sT.ap(), sstdT[:])

                            # v (natural [token, feature]; x8 chunk stationary)
                            with tc.tile_pool(name="ps_v", bufs=1,
                                              space="PSUM") as ps_v:
                                vacc = [ps_v.tile([128, 512], F32, tag="va",
                                                  bufs=8, name=f"va{half}_{u}")
                                        for u in range(TH // 128)]
                                for u in range(TH // 128):
                                    for c in range(KC2):
                                        nc.tensor.matmul(
                                            vacc[u][:],
                                            x8_t[:, 2 * c:2 * c + 2,
                                                 slice(half * TH + u * 128,
                                                       half * TH + (u + 1) * 128)],
                                            wv_t[:, c, :, :],
                                            start=(c == 0),
                                            stop=(c == KC2 - 1),
                                            perf_mode=DR)
                                    nc.vector.tensor_scalar_mul(
                                        vt[:, half * (TH // 128) + u, :],
                                        vacc[u][:], sstdT[:, u:u + 1])

                    if DEBUG_TAPS:
                        nc.sync.dma_start(dbg_kr.ap(), krope[0][:])
                        nc.sync.dma_start(dbg_vt.ap(), vt[:])
                    # ---------- Phase B: attention + out-proj ----------
                    with (
                        tc.tile_pool(name="pb", bufs=1) as pb,
                        tc.tile_pool(name="ps_sc", bufs=2, space="PSUM") as ps_sc,
                        tc.tile_pool(name="ps_av", bufs=1, space="PSUM") as ps_av,
                        tc.tile_pool(name="ps_po", bufs=2, space="PSUM") as ps_po,
                    ):
                        for tt in range(T // 512):
                            oT = [pb.tile([128, 2, 512], FP8, tag=f"oT{pr}",
                                          bufs=2, name=f"oT{pr}")
                                  for pr in range(2)]
                            for h in range(HEADS_PER_CORE):
                                out_ps = ps_av.tile([128, 512], F32, tag="av")
                                den_ps = ps_av.tile([128, 512], F32, tag="den")
                                for up in range(KC2):
                                    sc2 = ps_sc.tile([128, 2, 512], F32,
                                                     tag="sc")
                                    for i in range(2):
                                        nc.tensor.matmul(
                                            sc2[:, i, :],
                                            krope[h][:, ts(2 * up + i, 128)],
                                            qrope[h][:, ts(tt, 512)])
                                    at2 = pb.tile([128, 2, 512], FP8,
                                                  tag="at", bufs=4)
                                    nc.scalar.activation(at2[:], sc2[:],
                                                         Act.Exp,
                                                         bias=shift_t[:],
                                                         scale=SCALE)
                                    nc.tensor.matmul(
                                        out_ps[:],
                                        vt[:, 2 * up:2 * up + 2, ts(h, 128)],
                                        at2[:],
                                        start=(up == 0), stop=(up == KC2 - 1),
                                        perf_mode=DR)
                                    nc.tensor.matmul(
                                        den_ps[:], ones8[:], at2[:],
                                        start=(up == 0), stop=(up == KC2 - 1),
                                        perf_mode=DR)
                                    if DEBUG_TAPS and tt == 0 and h == 0 and up == 0:
                                        nc.sync.dma_start(dbg_at.ap(), at2[:])
                                rec = pb.tile([128, 512], F32, tag="rec",
                                              bufs=2)
                                if DEBUG_TAPS and tt == 0 and h == 0:
                                    dsb = pb.tile([128, 512], F32, tag="dsb")
                                    nc.vector.tensor_copy(dsb[:], den_ps[:])
                                    nc.sync.dma_start(dbg_den.ap(), dsb[:])
                                nc.vector.reciprocal(rec[:], den_ps[:])
                                nc.vector.tensor_mul(oT[h // 2][:, h % 2, :],
                                                     out_ps[:], rec[:])

                            for et in range(D_MODEL // 128):
                                pps = ps_po.tile([128, 512], F32, tag="po")
                                for pr in range(2):
                                    nc.tensor.matmul(
                                        pps[:],
                                        wo_t[:, pr, :, ts(et, 128)],
                                        oT[pr][:, :, :],
                                        start=(pr == 0), stop=(pr == 1),
                                        perf_mode=DR)
                                posb = pb.tile([128, 512], BF16, tag="posb",
                                               bufs=3)
                                nc.vector.tensor_copy(posb[:], pps[:])
                                nc.sync.dma_start(pod[ts(et, 128), ts(tt, 512)],
                                                  posb[:])

            if reps == 1:
                emit_body()
            else:
                with tc.For_i(0, reps, 1):
                    emit_body()

    nc.compile()
    _PROGRAMS[key] = nc
    return nc


def _rope_tables():
    import ml_dtypes
    inv_freq = 1.0 / (ROPE_BASE ** (np.arange(0, HEAD_DIM, 2,
                                              dtype=np.float32) / HEAD_DIM))
    t = np.arange(T, dtype=np.float32)
    freqs = np.outer(t, inv_freq)                      # (T, 64)
    emb = np.concatenate([freqs, freqs], axis=-1)      # (T, 128)
    cosT = np.ascontiguousarray(np.cos(emb).T)         # (128, T)
    sinT = np.ascontiguousarray(np.sin(emb).T)
    sinN = sinT.copy()
    sinN[0:64, :] *= -1.0                              # rotate-half sign
    return (cosT.astype(ml_dtypes.bfloat16),
            sinN.astype(ml_dtypes.bfloat16))


def _shard_inputs(x, norm_w, w_qkv, w_out):
    import ml_dtypes
    FP8 = ml_dtypes.float8_e4m3
    BF16 = ml_dtypes.bfloat16
    cosb, sinn = _rope_tables()
    nw = norm_w.astype(np.float32)
    ident = np.eye(128, dtype=np.float32)
    in_maps = []
    for c in range(N_CORES):
        b, hg = divmod(c, HG)
        rs = slice(hg * F, (hg + 1) * F)
        ks = slice(D_MODEL + hg * F, D_MODEL + (hg + 1) * F)
        vs = slice(2 * D_MODEL + hg * F, 2 * D_MODEL + (hg + 1) * F)
        xT = np.ascontiguousarray(x[b].T)                       # (D, T)
        x_kc = np.ascontiguousarray(
            xT.reshape(KC, 128, T).transpose(1, 0, 2))          # (128, KC, T)

        def wpack(rows):
            wT = (w_qkv[rows] * nw).T                           # (D, F)
            return np.ascontiguousarray(
                wT.reshape(KC2, 2, 128, F).transpose(2, 0, 1, 3)).astype(FP8)

        woT = w_out[:, rs].T                                    # (F, D)
        wo8 = np.ascontiguousarray(
            woT.reshape(2, 2, 128, D_MODEL).transpose(2, 0, 1, 3)).astype(FP8)
        in_maps.append({
            "x8": x_kc.astype(FP8),
            "xb": x_kc.astype(BF16),
            "wq8": wpack(rs),
            "wk8": wpack(ks),
            "wv8": wpack(vs),
            "wo8": wo8,
            "cosb": cosb,
            "sinn": sinn,
            "ident": ident,
        })
    return in_maps


def _gather(results, x):
    y = np.empty((B, T, D_MODEL), dtype=np.float32)
    for b in range(B):
        acc = np.asarray(x[b], dtype=np.float32).copy()
        for hg in range(HG):
            acc += results[b * HG + hg]["po"].astype(np.float32).T
        y[b] = acc
    return y


def run(x, norm_w, w_qkv, w_out, reps: int = 1):
    from concourse.bass_utils import run_bass_kernel_spmd

    nc = _build_program(reps)
    in_maps = _shard_inputs(x, norm_w, w_qkv, w_out)
    res = run_bass_kernel_spmd(nc, in_maps, core_ids=list(range(N_CORES)))
    return _gather(res.results, x)


def kernel(x, norm_w, w_qkv, w_out):
    last_err = None
    for _attempt in range(3):
        try:
            return run(np.asarray(x), np.asarray(norm_w), np.asarray(w_qkv),
                       np.asarray(w_out))
        except Exception as e:  # transient NRT_EXEC_UNIT_UNRECOVERABLE etc.
            last_err = e
    raise last_err

